# revision 2
# baseline (speedup 1.0000x reference)
"""Trainium2 Bass kernel v2 for nn_AttnBlock (B=1, C=128, H=32, W=128, 8 heads).

One head per NeuronCore. Main ideas vs the f32r baseline:

- All big matmuls run in fp8 with perf_mode=DoubleRow (0.5 cycles/row):
  * S^T tiles: lhsT = [A5*k_j | bias-block] pairs, rhs = [q_c | e-block]
    pairs. The bias-block/e-block slot injects the Schraudolph affine
    constant B5 so PSUM holds A5*S + B5 directly.
  * attn @ v runs transposed (et stationary, v moving, out = acc^T
    [l_q, 18]) so each matmul's out free dim is 18 and the epilogue
    needs no PE transpose and no acc evacuation.
- exp() is a Schraudolph bit-trick: byte = rint(A5*S + B5) stored as int8
  == fp8e5m2 bits of exp(4S - 2). Since PSUM already holds A5*S+B5, the
  "exp" is a pure fp32->int8 convert-copy, which both ScalarE (activation
  Copy) and VectorE (tensor_copy) execute bitwise-identically; tiles are
  split across both engines greedily (softmax renormalization then
  cancels the shared sawtooth bias exactly).
- v tiles [1|w_v] live in 32-wide blocks (DoubleRow pair stride %16==0).
"""

import math as _math

import numpy as np

N_CORES = 8
C = 128
H = 32
W = 128
L = H * W  # 4096
F = 8
D = 16
NCH = 8  # l_q chunks of 512
NKT = 32  # l_k tiles of 128

A4 = float(32.0 / _math.log(2))  # byte = A4*S + B4 -> e4m3 bits of exp(4S-3)
B4 = 21.5  # ~= 56 - 24/ln2 + 0.125; exact in bf16 AND fp32

# bf16 weight blob column layout
WB_KQ = 0      # [128, 64]: cols 0:16 w_q^T, 32:48 A4*w_k^T
WB_WV = 64     # [128, 18]: [0 | w_v^T ]  (col 0 zero keeps the ones column)
WB_WP = 82     # [128, 128]: w_proj^T
WB_ON128 = 210  # row0: 128 ones (v bias matmul lhsT)
WB_BV = 338    # row0: [1 | b_v] (18)
WB_BPC = 356   # all rows: b_proj column (partition w)
WB_B4 = 357    # all rows: B4 column
WB_BIA = 358   # row0: [bq(16) pad(16) A4*bk(16)] kq bias matmul lhsT
WB_ON512 = 422  # row0: 512 ones (kq bias matmul moving operand)
WB_W = 934

_CACHE = {}


def _build():
    import concourse.tile as tile
    from concourse import bacc, mybir

    f32 = mybir.dt.float32
    f32r = mybir.dt.float32r
    bf16 = mybir.dt.bfloat16
    i8 = mybir.dt.int8
    f8e4 = mybir.dt.float8e4
    f8e5 = mybir.dt.float8e5
    Copy = mybir.ActivationFunctionType.Copy
    Relu = mybir.ActivationFunctionType.Relu
    DR = mybir.MatmulPerfMode.DoubleRow
    mult = mybir.AluOpType.mult

    nc = bacc.Bacc("TRN2", target_bir_lowering=False, debug=False)

    x_d = nc.dram_tensor("x_cl", [C, L], bf16, kind="ExternalInput").ap()
    wb_d = nc.dram_tensor("wb", [C, WB_W], bf16, kind="ExternalInput").ap()
    fb_d = nc.dram_tensor("fb", [C, 1], f32r, kind="ExternalInput").ap()
    out_d = nc.dram_tensor("out", [L, D], f32, kind="ExternalOutput").ap()

    # greedy engine-balance state: [ACT, DVE] busy-ns estimates
    load = [0.0, 0.0]

    def pick(cost_a, cost_v):
        if load[0] + cost_a <= load[1] + cost_v:
            load[0] += cost_a
            return 0
        load[1] += cost_v
        return 1

    with tile.TileContext(nc) as tc:
        with (
            tc.tile_pool(name="consts", bufs=1) as consts,
            tc.tile_pool(name="etp", bufs=4) as etp,
            tc.tile_pool(name="epi", bufs=2) as epi,
        ):
            wb = consts.tile([C, WB_W], bf16)
            nc.scalar.dma_start(out=wb[:, 0:64], in_=wb_d[:, 0:64])
            nc.scalar.dma_start(out=wb[:, 64:], in_=wb_d[:, 64:])
            fb = consts.tile([C, 1], f32r)
            nc.scalar.dma_start(out=fb, in_=fb_d)
            b4_ap = wb[:, WB_B4:WB_B4 + 1]

            k_sb = consts.tile([16, NKT * 128], f8e4)
            # q_z: 8 blocks of [q_hi (512) | q_lo (512)]
            q_z = consts.tile([16, NCH * 1024], f8e4)

            x_sb = consts.tile([C, L], bf16)
            for cch in range(NCH):
                eng = nc.sync if cch % 2 == 0 else nc.gpsimd
                eng.dma_start(
                    out=x_sb[:, cch * 512:(cch + 1) * 512],
                    in_=x_d[:, cch * 512:(cch + 1) * 512],
                )

            v_sb = consts.tile([C, NKT * 32], f8e4)
            warm = consts.tile([1, 2], f32)
            nc.vector.memset(warm[:], 0.0)  # [1|v|pad14] blocks

            with (
                tc.tile_pool(name="sq", bufs=3, space="PSUM") as ps_sq,
                tc.tile_pool(name="acc", bufs=2, space="PSUM") as ps_acc,
            ):
                warm2 = epi.tile([1, 2], f32, tag="warm")
                nc.scalar.activation(out=warm2[:], in_=warm[:],
                                     func=Relu, bias=0.0, scale=1.0)

                def emit_kq(c):
                    kqps = ps_sq.tile([64, 512], f32, tag="squad")
                    nc.tensor.matmul(
                        kqps[:], wb[:, WB_KQ:WB_KQ + 64],
                        x_sb[:, c * 512:(c + 1) * 512], start=True, stop=False,
                        skip_group_check=True,
                    )
                    nc.tensor.matmul(
                        kqps[:], wb[0:1, WB_BIA:WB_BIA + 64],
                        wb[0:1, WB_ON512:WB_ON512 + 512], start=False,
                        stop=True, skip_group_check=True,
                    )
                    qhi = q_z[:, c * 1024:c * 1024 + 512]
                    nc.scalar.copy(qhi, kqps[0:16, :])
                    load[0] += 612.0
                    nc.vector.tensor_tensor(
                        out=q_z[:, c * 1024 + 512:(c + 1) * 1024],
                        in0=kqps[0:16, :], in1=qhi,
                        op=mybir.AluOpType.subtract,
                    )
                    load[1] += 658.0
                    if pick(612.0, 658.0) == 0:
                        nc.scalar.copy(
                            k_sb[:, c * 512:(c + 1) * 512], kqps[32:48, :])
                    else:
                        nc.vector.tensor_copy(
                            k_sb[:, c * 512:(c + 1) * 512], kqps[32:48, :])

                def emit_v_group(g):
                    # j = 8g..8g+7; uses x chunks 2g, 2g+1
                    vps = ps_sq.tile([C, 256], f32, tag="squad")
                    for u in range(8):
                        t = 8 * g + u
                        sl = slice(u * 32, u * 32 + 18)
                        nc.tensor.matmul(
                            vps[:, sl], wb[0:1, WB_ON128:WB_ON128 + 128],
                            wb[0:1, WB_BV:WB_BV + 18],
                            start=True, stop=False, skip_group_check=True,
                        )
                        nc.tensor.matmul(
                            vps[:, sl], x_sb[:, t * 128:(t + 1) * 128],
                            wb[:, WB_WV:WB_WV + 18],
                            start=False, stop=True, skip_group_check=True,
                        )
                    dst = v_sb[:, g * 256:(g + 1) * 256].rearrange(
                        "p (a b) -> p a b", a=8)[:, :, 0:18]
                    src = vps[:].rearrange("p (a b) -> p a b", a=8)[:, :, 0:18]
                    nc.vector.tensor_copy(dst, src)
                    load[1] += 275.0

                # j-groups per chunk: 16 pairs
                JGROUPS = [(2 * t, 2) for t in range(16)]

                def emit_group(c, accT, csl, gi):
                    qv = q_z[:, c * 1024:(c + 1) * 1024].rearrange(
                        "p (t n) -> p t n", t=2)
                    if True:
                        j0, glen = JGROUPS[gi]
                        squad = ps_sq.tile([128, 1024], f32, tag="squad")
                        for h in range(glen):
                            j = j0 + h
                            kv = k_sb[:, j * 128:(j + 1) * 128].unsqueeze(
                                1).broadcast_to([16, 2, 128])
                            nc.tensor.matmul(
                                squad[:, h * 512:(h + 1) * 512], kv, qv,
                                start=True, stop=True, perf_mode=DR,
                            )
                        et = etp.tile([128, 1024], i8, tag="et")
                        if pick(1038.0, 1192.0) == 0:
                            nc.scalar.activation(
                                out=et[:], in_=squad[:],
                                func=Relu, bias=b4_ap, scale=1.0,
                            )
                        else:
                            nc.vector.tensor_scalar(
                                out=et[:], in0=squad[:],
                                scalar1=B4, scalar2=0.0,
                                op0=mybir.AluOpType.add,
                                op1=mybir.AluOpType.max,
                            )
                        et5 = et[:].bitcast(f8e4)
                        start = gi == 0
                        stop = gi == len(JGROUPS) - 1
                        pair = et5[:].rearrange("p (t n) -> p t n", t=2)
                        vpair = v_sb[:, j0 * 32:j0 * 32 + 64].rearrange(
                            "p (t n) -> p t n", t=2)[:, :, 0:18]

                        def emit_ev(accT=accT, csl=csl, pair=pair,
                                    vpair=vpair, start=start, stop=stop):
                            for b in range(4):
                                # ONE psum start per accT tile (bank-wide
                                # pending-zero semantics).
                                nc.tensor.matmul(
                                    accT[:, csl + b * 18:csl + b * 18 + 18],
                                    pair[:, :, b * 128:(b + 1) * 128], vpair,
                                    start=(start and b == 0 and csl == 0),
                                    stop=(stop and b == 3 and csl == 72),
                                    perf_mode=DR, skip_group_check=True,
                                )
                        pending_ev.append(emit_ev)

                def emit_epilogue(t, accT, base):
                    # chunks 2t, 2t+1 -> out rows 1024t : 1024t+1024
                    accv = accT[:, base:base + 144].rearrange(
                        "p (a b) -> p a b", a=8)
                    rcp = epi.tile([128, 8], f32, tag="rcp")
                    nc.vector.reciprocal(rcp[:], accv[:, :, 0:1])
                    load[1] += 135.0
                    onorm = epi.tile([128, 128], bf16, tag="onorm")
                    nc.vector.tensor_tensor(
                        out=onorm[:].rearrange("p (a b) -> p a b", a=8),
                        in0=accv[:, :, 1:17],
                        in1=rcp[:].unsqueeze(2).broadcast_to([128, 8, 16]),
                        op=mult,
                    )
                    load[1] += 260.0
                    pps = ps_sq.tile([128, 128], f32, tag="squad")
                    for s in range(8):
                        nc.tensor.matmul(
                            pps[:, s * 16:(s + 1) * 16],
                            wb[:, WB_WP:WB_WP + 128],
                            onorm[:, s * 16:(s + 1) * 16],
                            start=(s == 0), stop=(s == 7),
                            skip_group_check=True,
                        )
                    osb = epi.tile([128, 128], f32, tag="osb")
                    bpc = fb[:, 0:1].bitcast(f32)
                    if pick(292.0, 258.0) == 0:
                        nc.scalar.activation(
                            out=osb[:], in_=pps[:], func=Copy, bias=0.0,
                            scale=1.0)
                        nc.vector.tensor_scalar_add(osb[:], osb[:], bpc)
                        load[1] += 258.0
                    else:
                        nc.vector.tensor_scalar_add(osb[:], pps[:], bpc)
                    od = out_d[t * 1024:(t + 1) * 1024, :].rearrange(
                        "(h p) d -> p h d", p=128)
                    eng = nc.sync if t == 3 else nc.gpsimd
                    eng.dma_start(
                        out=od, in_=osb[:].rearrange("p (h d) -> p h d", h=8))

                # ---- emission: all k/v production first (slice-level deps
                # let the main loop pipeline against it), then main chunks ----
                # Weave chunk-0 groups between kq/v prologue emissions so
                # the in-order engine queues start exp work early: group g of
                # chunk 0 needs k chunks <= (2g+1)//4 and vgrp <= (2g+1)//8.
                accT0 = ps_acc.tile([128, 144], f32, tag="accT")
                done = 0
                pending_ev = []

                def flush_ev(keep=1):
                    while len(pending_ev) > keep:
                        pending_ev.pop(0)()

                def emit_groups(c, accT, csl, upto):
                    nonlocal done
                    while done < upto:
                        emit_group(c, accT, csl, done)
                        done += 1
                        flush_ev(keep=1)

                # front-load kq 0-3 (pipelines against the x DMA stream),
                # then weave remaining kq/v with chunk-0 groups at ~6-group
                # evac lead so engine queues never starve the S matmuls.
                emit_kq(0)
                emit_kq(1)
                emit_kq(2)
                emit_v_group(0)
                emit_kq(3)
                emit_groups(0, accT0, 0, 2)
                emit_kq(4)
                emit_v_group(1)
                emit_groups(0, accT0, 0, 4)
                emit_kq(5)
                emit_groups(0, accT0, 0, 6)
                emit_kq(6)
                emit_v_group(2)
                emit_groups(0, accT0, 0, 8)
                emit_kq(7)
                emit_v_group(3)
                emit_groups(0, accT0, 0, 16)
                pending_epi = None
                accT = accT0
                for c in range(1, NCH):
                    t = c // 2
                    if c % 2 == 0:
                        accT = ps_acc.tile([128, 144], f32, tag="accT")
                    done = 0
                    emit_groups(c, accT, (c % 2) * 72, 16)
                    if c % 2 == 1:
                        flush_ev(keep=0)
                    if pending_epi is not None:
                        emit_epilogue(*pending_epi)
                        pending_epi = None
                    if c % 2 == 1:
                        pending_epi = (t, accT, 0)
                flush_ev(keep=0)
                emit_epilogue(*pending_epi)

    nc.compile()
    return nc


def _get_program():
    if "nc" not in _CACHE:
        _CACHE["nc"] = _build()
    return _CACHE["nc"]


def _make_in_maps(x, w_qkv, b_qkv, w_proj, b_proj):
    import ml_dtypes

    e4 = ml_dtypes.float8_e4m3
    bf = ml_dtypes.bfloat16

    x_cl = np.ascontiguousarray(
        np.asarray(x, dtype=np.float32).reshape(C, L).astype(bf))
    w_qkv = np.asarray(w_qkv, dtype=np.float32)
    b_qkv = np.asarray(b_qkv, dtype=np.float32)
    w_proj = np.asarray(w_proj, dtype=np.float32)
    b_proj = np.asarray(b_proj, dtype=np.float32)
    wpT = np.ascontiguousarray(w_proj.T)

    in_maps = []
    for i in range(N_CORES):
        rows_q = np.arange(D) * 24 + i * 3 + 0
        rows_k = rows_q + 1
        rows_v = rows_q + 2
        wb = np.zeros((C, WB_W), dtype=bf)
        wb[:, WB_KQ:WB_KQ + 16] = w_qkv[rows_q].T.astype(bf)
        wb[:, WB_KQ + 32:WB_KQ + 48] = (A4 * w_qkv[rows_k].T).astype(bf)
        wb[:, WB_WV + 1:WB_WV + 17] = w_qkv[rows_v].T.astype(bf)
        wb[:, WB_WP:WB_WP + 128] = wpT.astype(bf)
        wb[0, WB_ON128:WB_ON128 + 128] = bf(1.0)
        wb[0, WB_BV] = bf(1.0)
        wb[0, WB_BV + 1:WB_BV + 17] = b_qkv[rows_v].astype(bf)
        wb[0, WB_ON512:WB_ON512 + 512] = bf(1.0)
        wb[0, WB_BIA:WB_BIA + 16] = b_qkv[rows_q].astype(bf)
        wb[0, WB_BIA + 32:WB_BIA + 48] = (A4 * b_qkv[rows_k]).astype(bf)
        wb[:, WB_B4] = bf(B4)
        fbb = np.ascontiguousarray(b_proj.reshape(C, 1).astype(np.float32))
        in_maps.append({"x_cl": x_cl, "wb": wb, "fb": fbb})
    return in_maps


def _run(in_maps, trace=False):
    from concourse.bass_utils import run_bass_kernel_spmd

    nc = _get_program()
    return run_bass_kernel_spmd(nc, in_maps, list(range(N_CORES)), trace=trace)


def _assemble(results):
    out = np.empty((1, C, H, W), dtype=np.float32)
    for i in range(N_CORES):
        out[0, i * D:(i + 1) * D] = (
            results[i]["out"].reshape(H, W, D).transpose(2, 0, 1))
    return out


def kernel(x, w_qkv, b_qkv, w_proj, b_proj):
    in_maps = _make_in_maps(x, w_qkv, b_qkv, w_proj, b_proj)
    r = _run(in_maps, trace=False)
    return _assemble(r.results)


def kernel_with_timing(x, w_qkv, b_qkv, w_proj, b_proj):
    in_maps = _make_in_maps(x, w_qkv, b_qkv, w_proj, b_proj)
    try:
        r = _run(in_maps, trace=True)
        exec_ns = r.exec_time_ns
    except ModuleNotFoundError:
        r = _run(in_maps, trace=False)
        exec_ns = None
    if exec_ns is None:
        exec_ns = _CACHE.get("tlsim_ns")
        if exec_ns is None:
            from concourse.timeline_sim import TimelineSim

            exec_ns = int(TimelineSim(_get_program()).simulate())
            _CACHE["tlsim_ns"] = exec_ns
    return _assemble(r.results), exec_ns
